# revision 56
# baseline (speedup 1.0000x reference)
"""Trainium2 Bass kernel for nn_MiniARDepthTransformer (forward_step, S=T=1).

Math notes (exact simplifications of the reference):
  - Attention with a single key/query position: softmax over axis of size 1
    is identically 1.0, so attention(x, ctx) == (ctx @ Wv) @ Wo. The q/k
    projections never affect the output.
  - Self-attn:  x += rmsnorm(x, sa_n) @ (diag(sa_n) sa_v @ sa_o)
    Cross-attn: x += context @ (ca_v @ ca_o)   (independent of x!)
  - Norm weights fold into the following matmul weights; done on host in f64.

v3: every big matmul runs as fp8-e4m3 DoubleRow in THREE accumulating
terms  a_hi@W_hi + a_hi@W_lo + a_lo@W_hi  where X_hi = fp8(X) and
X_lo = fp8(X - X_hi).  The hi+lo pair carries ~7 mantissa bits, so the
result matches bf16-matmul accuracy (measured 6.9e-3 rel max err vs the
2e-2 budget) while costing 0.75x the PE cycles of bf16 (DoubleRow feeds
2 contraction rows per instruction at 0.5 cycles/output row).  Weight
and host-input splits (incl. the layer-0 self-attn xhat via an f64-
normalized embedding table) are free; on-device activation splits
(xhat, gu, ctx, final x) cost one bf16 staging op + fp8 cast + fp8
residual op, balanced across DVE/ACT/Pool.

Residual x stays bf16 (keeps DVE 2x fast modes); squared stats are fp8
so the partition-sum ones-matmul is DoubleRow too.  The final rmsnorm
folds into the unembed OUTPUT side: the unembed consumes split-fp8 raw
x batch-major, per-sample 1/rms comes from tiny [128,1] matmuls
(lhsT=sq, rhs=ones column), and lands as a per-partition scale on the
evacuation.  Output is DMA'd bf16 and upcast on host.

Sharding: pure data parallel over batch, 8 cores x 1024 rows; weights
replicated; no collectives.
"""

import os
import sys

import numpy as np

try:
    import concourse.bass  # noqa: F401
except ImportError:
    for _p in (os.environ.get("TRN_RL_REPO"), "/opt/trn_rl_repo",
               "/root/.axon_site/_ro/trn_rl_repo"):
        if _p and os.path.isdir(_p):
            sys.path.insert(0, _p)
            break

import ml_dtypes
from concourse import bacc, mybir, tile
from concourse import bass_utils

P = 128
D = 512
F = 1024
V = 2048
MD = 768
L = 4
EPS = 1e-6
N_CORES = 8
B = 8192
BC = B // N_CORES        # 1024 rows per core
BS = 512                 # batch chunk (columns) per norm/evac stream
NB = BC // BS            # 2 batch chunks
DK = D // P              # 4
FK = F // P              # 8
MK = MD // P             # 6
VK = V // BS             # 4
BT = BC // P             # 8 output row-tiles

BF16 = mybir.dt.bfloat16
F32 = mybir.dt.float32
FP8 = mybir.dt.float8e4
AF = mybir.ActivationFunctionType
ALU = mybir.AluOpType
DR = mybir.MatmulPerfMode.DoubleRow

WS = 64.0     # weight scale (all fp8 weight splits)
XS = 8.0      # normalized-activation scale (xhat, ctx, final x)
GS = 64.0     # gu scale

# Flexible elementwise ops: True -> GpSimd/Pool engine, False -> DVE.
# (Pool has no PSUM port; only all-SBUF ops may be assigned to it.)
POOL_Z = False        # norm z = x * binv muls (DVE 2x mode makes DVE best)
POOL_GU_EPS = True    # gu fp8 residual subs (fp8-out is 1x on DVE anyway)
POOL_NORM_EPS = True  # norm fp8 residual subs

_CACHE = {}


def _build_nc():
    nc = bacc.Bacc("TRN2", target_bir_lowering=False, debug=False,
                   num_devices=N_CORES)

    def din(name, shape, dt):
        return nc.dram_tensor(name, shape, dt, kind="ExternalInput")

    x0_d = din("x0", [P, DK, BC], BF16)
    xh0h_d = din("xh0h", [P, DK, BC], FP8)
    xh0l_d = din("xh0l", [P, DK, BC], FP8)
    mhh_d = din("mhh", [P, MK, BC], FP8)
    mhl_d = din("mhl", [P, MK, BC], FP8)
    wch_d = din("wch", [P, MK, D], FP8)
    wcl_d = din("wcl", [P, MK, D], FP8)
    bc_d = din("bcv", [P, DK], F32)
    wsah_d = din("wsah", [L, P, DK, D], FP8)
    wsal_d = din("wsal", [L, P, DK, D], FP8)
    wcah_d = din("wcah", [L, P, DK, D], FP8)
    wcal_d = din("wcal", [L, P, DK, D], FP8)
    wgh_d = din("wgh", [L, P, DK, F], FP8)
    wgl_d = din("wgl", [L, P, DK, F], FP8)
    wuh_d = din("wuh", [L, P, DK, F], FP8)
    wul_d = din("wul", [L, P, DK, F], FP8)
    wdh_d = din("wdh", [L, P, FK, D], FP8)
    wdl_d = din("wdl", [L, P, FK, D], FP8)
    woh_d = din("woh", [P, DK, V], FP8)
    wol_d = din("wol", [P, DK, V], FP8)
    out_d = nc.dram_tensor("out", [BC, V], BF16, kind="ExternalOutput")

    with tile.TileContext(nc) as tc:
        with (
            tc.tile_pool(name="consts", bufs=1) as consts,
            tc.tile_pool(name="persist", bufs=1) as persist,
            tc.tile_pool(name="wpool", bufs=2) as wpool,
            tc.tile_pool(name="norm", bufs=2) as norm,
            tc.tile_pool(name="sqp", bufs=3) as sqp,
            tc.tile_pool(name="xhh", bufs=4) as xhh,
            tc.tile_pool(name="xhl", bufs=4) as xhl,
            tc.tile_pool(name="gup", bufs=2) as gup,
            tc.tile_pool(name="sgp", bufs=3) as sgp,
            tc.tile_pool(name="stage", bufs=4) as stage,
            tc.tile_pool(name="tinyp", bufs=8) as tinyp,
            tc.tile_pool(name="ps_main", bufs=7, space="PSUM") as ps_main,
            tc.tile_pool(name="ps_ty", bufs=1, space="PSUM") as ps_ty,
        ):
            ones8 = consts.tile([P, 2, P], FP8)
            nc.vector.memset(ones8, 1.0)
            onecol8 = consts.tile([P, 2, 1], FP8)
            nc.vector.memset(onecol8, 1.0)
            eps_t8 = consts.tile([P, 1], F32)
            nc.vector.memset(eps_t8, EPS / (XS * XS))
            eps_tw = consts.tile([P, 1], F32)
            nc.vector.memset(eps_tw, EPS * (WS * XS) * (WS * XS))
            dmy_in = consts.tile([P, 1], F32)
            nc.vector.memset(dmy_in, 1.0)
            dmy_out = consts.tile([P, 1], F32)

            def preload_act(func, dep_ap=None):
                # Tiny activation that pins this table-set's load at a point
                # where the ACT engine is idle, not at the first real use.
                nc.scalar.activation(out=dmy_out[:],
                                     in_=dep_ap if dep_ap is not None
                                     else dmy_in[:],
                                     func=func, bias=eps_t8[:], scale=0.0)

            # DMA order: ctx deps first, then layer-0 weights; wout deferred.
            wch = consts.tile([P, MK, D], FP8)
            nc.sync.dma_start(out=wch[:], in_=wch_d.ap())
            wcl = consts.tile([P, MK, D], FP8)
            nc.sync.dma_start(out=wcl[:], in_=wcl_d.ap())
            bcT = consts.tile([P, DK], F32)
            nc.sync.dma_start(out=bcT[:], in_=bc_d.ap())
            mhh = persist.tile([P, MK, BC], FP8)
            mhl = persist.tile([P, MK, BC], FP8)
            x = persist.tile([P, DK, BC], BF16)
            for b in range(NB):
                s = slice(b * BS, (b + 1) * BS)
                nc.sync.dma_start(out=mhh[:, :, s], in_=mhh_d.ap()[:, :, s])
                nc.sync.dma_start(out=mhl[:, :, s], in_=mhl_d.ap()[:, :, s])
            xh0t = []
            for b in range(NB):
                s = slice(b * BS, (b + 1) * BS)
                h = xhh.tile([P, DK, BS], FP8, tag="xh")
                nc.sync.dma_start(out=h[:], in_=xh0h_d.ap()[:, :, s])
                lo = xhl.tile([P, DK, BS], FP8, tag="xl")
                nc.sync.dma_start(out=lo[:], in_=xh0l_d.ap()[:, :, s])
                xh0t.append((h, lo))
            for b in range(NB):
                s = slice(b * BS, (b + 1) * BS)
                nc.sync.dma_start(out=x[:, :, s], in_=x0_d.ap()[:, :, s])
            woh = consts.tile([P, DK, V], FP8)
            wol = consts.tile([P, DK, V], FP8)
            ctxh = persist.tile([P, DK, BC], FP8)
            ctxl = persist.tile([P, DK, BC], FP8)
            x8h = persist.tile([P, DK, BC], FP8)
            x8l = persist.tile([P, DK, BC], FP8)

            preload_act(AF.Sqrt)

            def bsl(b):
                return slice(b * BS, (b + 1) * BS)

            def psl(n):
                return slice(n * P, (n + 1) * P)

            def mm3(ps, ah, al, wh, wl, nk, n, stop=True, start=True):
                # ps (+)= (ah+al) @ (wh+wl)[:, :, psl(n)], 3 terms, DoubleRow
                # (the al@wl term is ~0.06% and dropped).
                cs = psl(n)
                first = True
                for (a, w) in ((ah, wh), (ah, wl), (al, wh)):
                    for k in range(nk // 2):
                        ksl = slice(2 * k, 2 * k + 2)
                        last = (a is al) and (k == nk // 2 - 1)
                        nc.tensor.matmul(
                            ps[:], w[:, ksl, cs], a[:, ksl, :],
                            start=(start and first), stop=(stop and last),
                            perf_mode=DR)
                        first = False

            # ---- context = XS * (mh @ Wc + bc), split fp8 dim-major ----
            ctxz = persist.tile([P, DK, BC], BF16)
            for b in range(NB):
                psc = []
                for n in range(DK):
                    psn = ps_main.tile([P, BS], F32, tag="m")
                    psc.append(psn)
                # term-outer: all hi@hi first so PE starts as soon as
                # wch + mhh(b) land, before the lo DMAs finish
                for (a, w) in ((mhh, wch), (mhh, wcl), (mhl, wch)):
                    for n in range(DK):
                        for k in range(MK // 2):
                            ksl = slice(2 * k, 2 * k + 2)
                            nc.tensor.matmul(
                                psc[n][:], w[:, ksl, psl(n)],
                                a[:, ksl, bsl(b)],
                                start=(a is mhh and w is wch and k == 0),
                                stop=(a is mhl and k == MK // 2 - 1),
                                perf_mode=DR)
                for n in range(DK):
                    nc.scalar.activation(out=ctxz[:, n, bsl(b)],
                                         in_=psc[n][:],
                                         func=AF.Identity,
                                         bias=bcT[:, n:n + 1], scale=XS / WS)
            for b in range(NB):
                nc.vector.tensor_copy(out=ctxh[:, :, bsl(b)],
                                      in_=ctxz[:, :, bsl(b)])
                nc.gpsimd.tensor_sub(out=ctxl[:, :, bsl(b)],
                                     in0=ctxz[:, :, bsl(b)],
                                     in1=ctxh[:, :, bsl(b)])

            def rmsnorm_a(b):
                # phase A: sq = fp8(x*x) on ACT (no PE instruction, so it
                # can be emitted right after the stream's evacs).
                sq = sqp.tile([P, DK, BS], FP8, tag="sq")
                nc.scalar.activation(out=sq[:], in_=x[:, :, bsl(b)],
                                     func=AF.Square)
                return sq

            def sq_alloc():
                sq = sqp.tile([P, DK, BS], FP8, tag="sq")
                return sq

            def sq_half(b, n, sq):
                nc.scalar.activation(out=sq[:, n], in_=x[:, n, bsl(b)],
                                     func=AF.Square)

            def rmsnorm_b(b, sq, pool=False):
                # phase B: ss = DoubleRow ones-matmul (emit where sq is
                # already done, or PE head-of-line blocks); sqrt on ACT;
                # binv = XS*rsqrt via DVE recip -> bf16; z = x*binv bf16;
                # hi = fp8(z); lo = fp8(z - hi), per k-pair so the first
                # matmul terms unblock as early as possible.
                bs = bsl(b)
                z = norm.tile([P, DK, BS], BF16, tag="z")
                binv = norm.tile([P, BS], F32, tag="binv")
                binvb = norm.tile([P, BS], BF16, tag="binvb")
                srt = norm.tile([P, BS], F32, tag="srt")
                xh = xhh.tile([P, DK, BS], FP8, tag="xh")
                xl = xhl.tile([P, DK, BS], FP8, tag="xl")
                ss = ps_main.tile([P, BS], F32, tag="m")
                for k in range(DK // 2):
                    nc.tensor.matmul(ss[:], ones8[:],
                                     sq[:, 2 * k:2 * k + 2, :],
                                     start=(k == 0), stop=(k == DK // 2 - 1),
                                     perf_mode=DR)
                nc.scalar.activation(out=srt[:], in_=ss[:], func=AF.Sqrt,
                                     bias=eps_t8[:],
                                     scale=1.0 / (D * XS * XS))
                nc.vector.reciprocal_approx_fast(out=binv[:], in_=srt[:])
                eng = nc.gpsimd if pool else nc.vector
                eng.tensor_copy(out=binvb[:], in_=binv[:])
                for k in range(DK // 2):
                    ksl = slice(2 * k, 2 * k + 2)
                    for d in (2 * k, 2 * k + 1):
                        eng.tensor_mul(out=z[:, d], in0=x[:, d, bs],
                                       in1=binvb[:])
                    eng.tensor_copy(out=xh[:, ksl], in_=z[:, ksl])
                    eng.tensor_sub(out=xl[:, ksl], in0=z[:, ksl],
                                   in1=xh[:, ksl])
                return xh, xl

            for i in range(L):
                if i == L - 1:
                    nc.sync.dma_start(out=woh[:], in_=woh_d.ap())
                    nc.sync.dma_start(out=wol[:], in_=wol_d.ap())
                wsah = wpool.tile([P, DK, D], FP8, tag="wsah")
                nc.sync.dma_start(out=wsah[:], in_=wsah_d.ap()[i])
                wcah = wpool.tile([P, DK, D], FP8, tag="wcah")
                nc.sync.dma_start(out=wcah[:], in_=wcah_d.ap()[i])
                wsal = wpool.tile([P, DK, D], FP8, tag="wsal")
                nc.sync.dma_start(out=wsal[:], in_=wsal_d.ap()[i])
                wcal = wpool.tile([P, DK, D], FP8, tag="wcal")
                nc.sync.dma_start(out=wcal[:], in_=wcal_d.ap()[i])
                wgh = wpool.tile([P, DK, F], FP8, tag="wgh")
                nc.sync.dma_start(out=wgh[:], in_=wgh_d.ap()[i])
                wgl = wpool.tile([P, DK, F], FP8, tag="wgl")
                nc.sync.dma_start(out=wgl[:], in_=wgl_d.ap()[i])
                wuh = wpool.tile([P, DK, F], FP8, tag="wuh")
                nc.sync.dma_start(out=wuh[:], in_=wuh_d.ap()[i])
                wul = wpool.tile([P, DK, F], FP8, tag="wul")
                nc.sync.dma_start(out=wul[:], in_=wul_d.ap()[i])
                wdh = wpool.tile([P, FK, D], FP8, tag="wdh")
                nc.sync.dma_start(out=wdh[:], in_=wdh_d.ap()[i])
                wdl = wpool.tile([P, FK, D], FP8, tag="wdl")
                nc.sync.dma_start(out=wdl[:], in_=wdl_d.ap()[i])

                # ---- skewed two-stream layer body ----
                # Emission (= PE SEQ) order staggers the b0/b1 streams so
                # each serial norm / gu-split chain on ACT/DVE/Pool drains
                # under the OTHER stream's matmuls:
                #   sa/ca b0 | sa/ca b1      (ffn-norm b0 in its shadow)
                #   g/u  b0  | g/u  b1       (ffn-norm b1 / gu-gen b0)
                #   down b0  | down b1       (gu-gen b1 / next sa-norm b0)
                # Next-layer sa-norms are emitted right after each stream's
                # down evacs, so they drain during the remaining stream work.
                def ca_pass(b):
                    pss = []
                    for n in range(DK):
                        ps = ps_main.tile([P, BS], F32, tag="m")
                        pss.append(ps)
                        mm3(ps, ctxh[:, :, bsl(b)], ctxl[:, :, bsl(b)],
                            wcah, wcal, DK, n, stop=False)
                    return pss

                def sa_pass(b, pss, sq=None):
                    xh = xh_sa[b]
                    for n in range(DK):
                        mm3(pss[n], xh[0], xh[1], wsah, wsal, DK, n,
                            start=False)
                        xsl = x[:, n, bsl(b)]
                        nc.vector.scalar_tensor_tensor(
                            out=xsl, in0=pss[n][:], scalar=1.0 / (WS * XS),
                            in1=xsl, op0=ALU.mult, op1=ALU.add)
                        if sq is not None:
                            sq_half(b, n, sq)

                def gu_stage(b, mid=None):
                    guh = gup.tile([P, FK, BS], FP8, tag="guh")
                    for f in range(FK):
                        if f == FK // 2 and mid is not None:
                            mid()
                        psg = ps_main.tile([P, BS], F32, tag="m")
                        psu = ps_main.tile([P, BS], F32, tag="m")
                        mm3(psg, xh_ffn[b][0], xh_ffn[b][1], wgh, wgl, DK, f)
                        mm3(psu, xh_ffn[b][0], xh_ffn[b][1], wuh, wul, DK, f)
                        sg = sgp.tile([P, BS], BF16, tag="sg")
                        nc.scalar.activation(out=sg[:], in_=psg[:],
                                             func=AF.Silu,
                                             scale=1.0 / (WS * XS))
                        # u evacs on ACT so the gub product is an all-bf16
                        # SBUF DVE mul (2x fast mode), then split to fp8
                        ub = sgp.tile([P, BS], BF16, tag="ub")
                        nc.scalar.activation(out=ub[:], in_=psu[:],
                                             func=AF.Identity,
                                             scale=GS / (WS * XS))
                        gub = sgp.tile([P, BS], BF16, tag="gub")
                        nc.vector.tensor_mul(out=gub[:], in0=sg[:],
                                             in1=ub[:])
                        heng = nc.gpsimd if b == 0 else nc.vector
                        heng.tensor_copy(out=guh[:, f], in_=gub[:])
                    return guh, None, sg

                def down(b, guh, gul, sq=None):
                    # 2-term: gu_lo dropped (see gu_stage)
                    for n in range(DK):
                        ps = ps_main.tile([P, BS], F32, tag="m")
                        cs = psl(n)
                        first = True
                        for w in (wdh, wdl):
                            for k in range(FK // 2):
                                ksl = slice(2 * k, 2 * k + 2)
                                last = (w is wdl) and (k == FK // 2 - 1)
                                nc.tensor.matmul(
                                    ps[:], w[:, ksl, cs], guh[:, ksl, :],
                                    start=first, stop=last, perf_mode=DR)
                                first = False
                        xsl = x[:, n, bsl(b)]
                        nc.vector.scalar_tensor_tensor(
                            out=xsl, in0=ps[:], scalar=1.0 / (WS * GS),
                            in1=xsl, op0=ALU.mult, op1=ALU.add)
                        if sq is not None:
                            sq_half(b, n, sq)

                sq0 = sq_alloc()
                sq1 = sq_alloc()
                if i == 0:
                    xh_sa = xh0t
                    pss0 = ca_pass(0)
                    sa_pass(0, pss0, sq=sq0)
                    pss1 = ca_pass(1)
                    sa_pass(1, pss1, sq=sq1)
                    xh_ffn = [rmsnorm_b(0, sq0), None]
                else:
                    # xh_sa[0] norm chain (emitted in the prev layer tail)
                    # drains under ca_pass(0); xh_sa[1] under sa_pass(0)
                    # + ca_pass(1).
                    pss0 = ca_pass(0)
                    xh_sa[1] = rmsnorm_b(1, sqn1)
                    sa_pass(0, pss0, sq=sq0)
                    pss1 = ca_pass(1)
                    sa_pass(1, pss1, sq=sq1)
                    xh_ffn = [rmsnorm_b(0, sq0), None]
                xh_ffn[1] = rmsnorm_b(1, sq1)
                if i == 0:
                    preload_act(AF.Silu, xh_ffn[0][0][:, 0, 0:1])
                gu0 = gu_stage(0)
                gu1 = gu_stage(1)
                if i < L - 1:
                    sqn0 = sq_alloc()
                    down(0, gu0[0], gu0[1], sq=sqn0)
                    # b0's next sa-norm: sq halves landed inline with the
                    # down(0) evacs, so its ss barely stalls PE here and the
                    # DVE chain drains under down(1).
                    xh_sa = [rmsnorm_b(0, sqn0), None]
                else:
                    down(0, gu0[0], gu0[1])
                    # Final-norm sqrt table, pinned while PE runs down b1.
                    preload_act(AF.Sqrt, gu1[2][:, 0:1])
                down(1, gu1[0], gu1[1])
                if i < L - 1:
                    sqn1 = rmsnorm_a(1)

            # ---- final rmsnorm folded into the unembed output side ----
            # logits[s, v] = (x8[:, s] . wout[:, v]) * bcol[s] with
            # x8 = split-fp8(XS * x) and bcol = rsqrt(mean x^2 + eps)/(WS*XS)
            # (batch-major [128,1] via tiny sq-matmuls, no transpose).
            bcols = []
            for b in range(NB):
                bs = bsl(b)
                z8 = norm.tile([P, DK, BS], BF16, tag="z8")
                nc.scalar.activation(out=z8[:], in_=x[:, :, bs],
                                     func=AF.Identity, scale=XS)
                nc.vector.tensor_copy(out=x8h[:, :, bs], in_=z8[:])
                nc.gpsimd.tensor_sub(out=x8l[:, :, bs], in0=z8[:],
                                     in1=x8h[:, :, bs])
                sq = sqp.tile([P, DK, BS], FP8, tag="sq")
                nc.scalar.activation(out=sq[:], in_=x[:, :, bs],
                                     func=AF.Square)
                for t in range(BS // P):
                    bt = b * (BS // P) + t
                    ssb = ps_ty.tile([P, 1], F32, tag="tiny")
                    for k in range(DK // 2):
                        nc.tensor.matmul(
                            ssb[:], sq[:, 2 * k:2 * k + 2, t * P:(t + 1) * P],
                            onecol8[:], start=(k == 0),
                            stop=(k == DK // 2 - 1), perf_mode=DR)
                    srtc = tinyp.tile([P, 1], F32, tag="srtc")
                    # srt = WS*XS*sqrt(ms+eps); bcol = rsqrt/(WS*XS)
                    nc.scalar.activation(out=srtc[:], in_=ssb[:],
                                         func=AF.Sqrt, bias=eps_tw[:],
                                         scale=WS * XS * WS * XS / D)
                    bcol = tinyp.tile([P, 1], F32, tag="bcol")
                    nc.vector.reciprocal_approx_fast(out=bcol[:],
                                                     in_=srtc[:])
                    bcols.append(bcol)

            for bt in range(BT):
                for v in range(VK):
                    ps = ps_main.tile([P, BS], F32, tag="m")
                    first = True
                    for (a, w) in ((x8h, woh), (x8h, wol), (x8l, woh)):
                        for k in range(DK // 2):
                            ksl = slice(2 * k, 2 * k + 2)
                            last = (a is x8l) and (k == DK // 2 - 1)
                            nc.tensor.matmul(
                                ps[:], a[:, ksl, psl(bt)],
                                w[:, ksl, bsl(v)],
                                start=first, stop=last, perf_mode=DR)
                            first = False
                    if v % 2 == 0:
                        st = stage.tile([P, 2, BS], BF16, tag="st")
                        nc.scalar.activation(out=st[:, 0], in_=ps[:],
                                             func=AF.Identity,
                                             scale=bcols[bt][:])
                    else:
                        # in1 ignored by bypass; must be SBUF, not PSUM
                        nc.vector.scalar_tensor_tensor(
                            out=st[:, 1], in0=ps[:], scalar=bcols[bt][:],
                            in1=x[:, 0, 0:BS], op0=ALU.mult, op1=ALU.bypass)
                        # one wide DMA per v-pair halves HWDGE setups
                        nc.sync.dma_start(
                            out=out_d.ap()[psl(bt),
                                           (v - 1) * BS:(v + 1) * BS],
                            in_=st[:])

    nc.finalize()
    return nc


def _get_nc():
    if "nc" not in _CACHE:
        _CACHE["nc"] = _build_nc()
    return _CACHE["nc"]


def _bf16(a):
    return np.ascontiguousarray(a).astype(ml_dtypes.bfloat16)


_F8NP = mybir.dt.np(FP8)


def _split8(a):
    hi = np.ascontiguousarray(a).astype(_F8NP)
    lo = np.ascontiguousarray(a - hi.astype(np.float64)).astype(_F8NP)
    return hi, lo


def _kxn(w):
    # [K, N] f64 -> [P, K//P, N]: [:, k, n0:n0+128] is a natural lhsT chunk.
    K, N = w.shape
    return np.ascontiguousarray(w.reshape(K // P, P, N).transpose(1, 0, 2))


def _dim_major(a):
    # [B, K] -> per-core [core, P, K//P, BC]
    K = a.shape[1]
    return a.reshape(N_CORES, BC, K // P, P).transpose(0, 3, 2, 1)


def _prep_inputs(inputs):
    f = {k: np.asarray(v) for k, v in inputs.items()}
    prev = f["prev_tokens"].reshape(-1).astype(np.int64)
    emb = f["emb"].astype(np.float64)
    mhf = f["main_hidden"].reshape(B, MD).astype(np.float64)

    # Layer-0 sa xhat: exact f64 normalization of the embedding TABLE,
    # split to fp8 per-row, then gathered (split before gather == after).
    nemb = emb * (XS / np.sqrt((emb * emb).mean(axis=1, keepdims=True) + EPS))
    nembh, nembl = _split8(nemb)
    x0c = _dim_major(emb[prev])
    xh0hc = _dim_major(nembh.astype(np.float64)[prev])
    xh0lc = _dim_major(nembl.astype(np.float64)[prev])
    mhc = _dim_major(mhf)

    f64 = lambda k: f[k].astype(np.float64)
    sa_n, sa_v, sa_o = f64("sa_n"), f64("sa_v"), f64("sa_o")
    ca_v, ca_o = f64("ca_v"), f64("ca_o")
    ffn_n, w_g, w_u, w_d = f64("ffn_n"), f64("w_g"), f64("w_u"), f64("w_d")

    def wsplit(ws):
        hs, ls = zip(*(_split8(w) for w in ws))
        return np.stack(hs), np.stack(ls)

    wsah, wsal = wsplit([_kxn(WS * (sa_n[i][:, None] * sa_v[i]) @ sa_o[i])
                         for i in range(L)])
    wcah, wcal = wsplit([_kxn(WS * ca_v[i] @ ca_o[i]) for i in range(L)])
    wgh, wgl = wsplit([_kxn(WS * ffn_n[i][:, None] * w_g[i])
                       for i in range(L)])
    wuh, wul = wsplit([_kxn(WS * ffn_n[i][:, None] * w_u[i])
                       for i in range(L)])
    wdh, wdl = wsplit([_kxn(WS * w_d[i]) for i in range(L)])
    woh, wol = _split8(_kxn(WS * f64("final_n")[:, None] * f64("Wout")))
    wch, wcl = _split8(_kxn(WS * f64("Wc")))
    bcv = np.ascontiguousarray(
        XS * f["bc"].astype(np.float64).reshape(DK, P).T)

    shared = {
        "wch": wch, "wcl": wcl, "bcv": bcv.astype(np.float32),
        "wsah": wsah, "wsal": wsal, "wcah": wcah, "wcal": wcal,
        "wgh": wgh, "wgl": wgl, "wuh": wuh, "wul": wul,
        "wdh": wdh, "wdl": wdl, "woh": woh, "wol": wol,
    }
    in_maps = []
    for c in range(N_CORES):
        m = dict(shared)
        m["x0"] = _bf16(x0c[c])
        m["mhh"], m["mhl"] = _split8(mhc[c])
        m["xh0h"] = xh0hc[c].astype(_F8NP)
        m["xh0l"] = xh0lc[c].astype(_F8NP)
        in_maps.append(m)
    return in_maps


def _run(inputs, trace=False, **kw):
    nc = _get_nc()
    in_maps = _prep_inputs(inputs)
    res = bass_utils.run_bass_kernel_spmd(
        nc, in_maps, core_ids=list(range(N_CORES)), trace=trace, **kw)
    out = np.concatenate([res.results[c]["out"] for c in range(N_CORES)],
                         axis=0)
    return out.astype(np.float32), res


def kernel(**inputs) -> np.ndarray:
    out, _ = _run(inputs, trace=False)
    return out


# revision 62
# speedup vs baseline: 1.0021x; 1.0021x over previous
"""Trainium2 Bass kernel for nn_MiniARDepthTransformer (forward_step, S=T=1).

Math notes (exact simplifications of the reference):
  - Attention with a single key/query position: softmax over axis of size 1
    is identically 1.0, so attention(x, ctx) == (ctx @ Wv) @ Wo. The q/k
    projections never affect the output.
  - Self-attn:  x += rmsnorm(x, sa_n) @ (diag(sa_n) sa_v @ sa_o)
    Cross-attn: x += context @ (ca_v @ ca_o)   (independent of x!)
  - Norm weights fold into the following matmul weights; done on host in f64.

v3: every big matmul runs as fp8-e4m3 DoubleRow in THREE accumulating
terms  a_hi@W_hi + a_hi@W_lo + a_lo@W_hi  where X_hi = fp8(X) and
X_lo = fp8(X - X_hi).  The hi+lo pair carries ~7 mantissa bits, so the
result matches bf16-matmul accuracy (measured 6.9e-3 rel max err vs the
2e-2 budget) while costing 0.75x the PE cycles of bf16 (DoubleRow feeds
2 contraction rows per instruction at 0.5 cycles/output row).  Weight
and host-input splits (incl. the layer-0 self-attn xhat via an f64-
normalized embedding table) are free; on-device activation splits
(xhat, gu, ctx, final x) cost one bf16 staging op + fp8 cast + fp8
residual op, balanced across DVE/ACT/Pool.

Residual x stays bf16 (keeps DVE 2x fast modes); squared stats are fp8
so the partition-sum ones-matmul is DoubleRow too.  The final rmsnorm
folds into the unembed OUTPUT side: the unembed consumes split-fp8 raw
x batch-major, per-sample 1/rms comes from tiny [128,1] matmuls
(lhsT=sq, rhs=ones column), and lands as a per-partition scale on the
evacuation.  Output is DMA'd bf16 and upcast on host.

Sharding: pure data parallel over batch, 8 cores x 1024 rows; weights
replicated; no collectives.
"""

import os
import sys

import numpy as np

try:
    import concourse.bass  # noqa: F401
except ImportError:
    for _p in (os.environ.get("TRN_RL_REPO"), "/opt/trn_rl_repo",
               "/root/.axon_site/_ro/trn_rl_repo"):
        if _p and os.path.isdir(_p):
            sys.path.insert(0, _p)
            break

import ml_dtypes
from concourse import bacc, mybir, tile
from concourse import bass_utils

P = 128
D = 512
F = 1024
V = 2048
MD = 768
L = 4
EPS = 1e-6
N_CORES = 8
B = 8192
BC = B // N_CORES        # 1024 rows per core
BS = 512                 # batch chunk (columns) per norm/evac stream
NB = BC // BS            # 2 batch chunks
DK = D // P              # 4
FK = F // P              # 8
MK = MD // P             # 6
VK = V // BS             # 4
BT = BC // P             # 8 output row-tiles

BF16 = mybir.dt.bfloat16
F32 = mybir.dt.float32
FP8 = mybir.dt.float8e4
AF = mybir.ActivationFunctionType
ALU = mybir.AluOpType
DR = mybir.MatmulPerfMode.DoubleRow

WS = 64.0     # weight scale (all fp8 weight splits)
XS = 8.0      # normalized-activation scale (xhat, ctx, final x)
GS = 64.0     # gu scale

# Flexible elementwise ops: True -> GpSimd/Pool engine, False -> DVE.
# (Pool has no PSUM port; only all-SBUF ops may be assigned to it.)
POOL_Z = False        # norm z = x * binv muls (DVE 2x mode makes DVE best)
POOL_GU_EPS = True    # gu fp8 residual subs (fp8-out is 1x on DVE anyway)
POOL_NORM_EPS = True  # norm fp8 residual subs

_CACHE = {}


def _build_nc():
    nc = bacc.Bacc("TRN2", target_bir_lowering=False, debug=False,
                   num_devices=N_CORES)

    def din(name, shape, dt):
        return nc.dram_tensor(name, shape, dt, kind="ExternalInput")

    x0_d = din("x0", [P, DK, BC], BF16)
    xh0h_d = din("xh0h", [P, DK, BC], FP8)
    xh0l_d = din("xh0l", [P, DK, BC], FP8)
    mhh_d = din("mhh", [P, MK, BC], FP8)
    mhl_d = din("mhl", [P, MK, BC], FP8)
    wch_d = din("wch", [P, MK, D], FP8)
    wcl_d = din("wcl", [P, MK, D], FP8)
    bc_d = din("bcv", [P, DK], F32)
    wsah_d = din("wsah", [L, P, DK, D], FP8)
    wsal_d = din("wsal", [L, P, DK, D], FP8)
    wcah_d = din("wcah", [L, P, DK, D], FP8)
    wcal_d = din("wcal", [L, P, DK, D], FP8)
    wgh_d = din("wgh", [L, P, DK, F], FP8)
    wgl_d = din("wgl", [L, P, DK, F], FP8)
    wuh_d = din("wuh", [L, P, DK, F], FP8)
    wul_d = din("wul", [L, P, DK, F], FP8)
    wdh_d = din("wdh", [L, P, FK, D], FP8)
    wdl_d = din("wdl", [L, P, FK, D], FP8)
    woh_d = din("woh", [P, DK, V], FP8)
    wol_d = din("wol", [P, DK, V], FP8)
    out_d = nc.dram_tensor("out", [BC, V], BF16, kind="ExternalOutput")

    with tile.TileContext(nc) as tc:
        with (
            tc.tile_pool(name="consts", bufs=1) as consts,
            tc.tile_pool(name="persist", bufs=1) as persist,
            tc.tile_pool(name="wpool", bufs=2) as wpool,
            tc.tile_pool(name="norm", bufs=2) as norm,
            tc.tile_pool(name="sqp", bufs=3) as sqp,
            tc.tile_pool(name="xhh", bufs=4) as xhh,
            tc.tile_pool(name="xhl", bufs=4) as xhl,
            tc.tile_pool(name="gup", bufs=2) as gup,
            tc.tile_pool(name="sgp", bufs=3) as sgp,
            tc.tile_pool(name="stage", bufs=4) as stage,
            tc.tile_pool(name="tinyp", bufs=8) as tinyp,
            tc.tile_pool(name="ps_main", bufs=7, space="PSUM") as ps_main,
            tc.tile_pool(name="ps_ty", bufs=1, space="PSUM") as ps_ty,
        ):
            ones8 = consts.tile([P, 2, P], FP8)
            nc.vector.memset(ones8, 1.0)
            onecol8 = consts.tile([P, 2, 1], FP8)
            nc.vector.memset(onecol8, 1.0)
            eps_t8 = consts.tile([P, 1], F32)
            nc.vector.memset(eps_t8, EPS / (XS * XS))
            eps_tw = consts.tile([P, 1], F32)
            nc.vector.memset(eps_tw, EPS * (WS * XS) * (WS * XS))
            dmy_in = consts.tile([P, 1], F32)
            nc.vector.memset(dmy_in, 1.0)
            dmy_out = consts.tile([P, 1], F32)

            def preload_act(func, dep_ap=None):
                # Tiny activation that pins this table-set's load at a point
                # where the ACT engine is idle, not at the first real use.
                nc.scalar.activation(out=dmy_out[:],
                                     in_=dep_ap if dep_ap is not None
                                     else dmy_in[:],
                                     func=func, bias=eps_t8[:], scale=0.0)

            # DMA order: ctx deps first, then layer-0 weights; wout deferred.
            wch = consts.tile([P, MK, D], FP8)
            nc.sync.dma_start(out=wch[:], in_=wch_d.ap())
            wcl = consts.tile([P, MK, D], FP8)
            nc.sync.dma_start(out=wcl[:], in_=wcl_d.ap())
            bcT = consts.tile([P, DK], F32)
            nc.sync.dma_start(out=bcT[:], in_=bc_d.ap())
            mhh = persist.tile([P, MK, BC], FP8)
            mhl = persist.tile([P, MK, BC], FP8)
            x = persist.tile([P, DK, BC], BF16)
            # b0's mh halves ride the ACT HWDGE queue so they transfer
            # concurrently with wch/wcl on the SP queue (only 2 DMAs of
            # config time on ACT, long before its first compute).
            nc.scalar.dma_start(out=mhh[:, :, 0:BS],
                                in_=mhh_d.ap()[:, :, 0:BS])
            nc.scalar.dma_start(out=mhl[:, :, 0:BS],
                                in_=mhl_d.ap()[:, :, 0:BS])
            nc.sync.dma_start(out=mhh[:, :, BS:BC],
                              in_=mhh_d.ap()[:, :, BS:BC])
            nc.sync.dma_start(out=mhl[:, :, BS:BC],
                              in_=mhl_d.ap()[:, :, BS:BC])
            xh0t = []
            for b in range(NB):
                s = slice(b * BS, (b + 1) * BS)
                eng = nc.scalar if b == 0 else nc.sync
                h = xhh.tile([P, DK, BS], FP8, tag="xh")
                eng.dma_start(out=h[:], in_=xh0h_d.ap()[:, :, s])
                lo = xhl.tile([P, DK, BS], FP8, tag="xl")
                eng.dma_start(out=lo[:], in_=xh0l_d.ap()[:, :, s])
                xh0t.append((h, lo))
            for b in range(NB):
                s = slice(b * BS, (b + 1) * BS)
                nc.sync.dma_start(out=x[:, :, s], in_=x0_d.ap()[:, :, s])
            woh = consts.tile([P, DK, V], FP8)
            wol = consts.tile([P, DK, V], FP8)
            ctxh = persist.tile([P, DK, BC], FP8)
            ctxl = persist.tile([P, DK, BC], FP8)
            x8h = persist.tile([P, DK, BC], FP8)
            x8l = persist.tile([P, DK, BC], FP8)

            preload_act(AF.Sqrt)

            def bsl(b):
                return slice(b * BS, (b + 1) * BS)

            def psl(n):
                return slice(n * P, (n + 1) * P)

            def mm3(ps, ah, al, wh, wl, nk, n, stop=True, start=True):
                # ps (+)= (ah+al) @ (wh+wl)[:, :, psl(n)], 3 terms, DoubleRow
                # (the al@wl term is ~0.06% and dropped).
                cs = psl(n)
                first = True
                for (a, w) in ((ah, wh), (ah, wl), (al, wh)):
                    for k in range(nk // 2):
                        ksl = slice(2 * k, 2 * k + 2)
                        last = (a is al) and (k == nk // 2 - 1)
                        nc.tensor.matmul(
                            ps[:], w[:, ksl, cs], a[:, ksl, :],
                            start=(start and first), stop=(stop and last),
                            perf_mode=DR)
                        first = False

            # ---- context = XS * (mh @ Wc + bc), split fp8 dim-major ----
            ctxz = persist.tile([P, DK, BC], BF16)
            for b in range(NB):
                psc = []
                for n in range(DK):
                    psn = ps_main.tile([P, BS], F32, tag="m")
                    psc.append(psn)
                # term-outer: all hi@hi first so PE starts as soon as
                # wch + mhh(b) land, before the lo DMAs finish
                for (a, w) in ((mhh, wch), (mhh, wcl), (mhl, wch)):
                    for n in range(DK):
                        for k in range(MK // 2):
                            ksl = slice(2 * k, 2 * k + 2)
                            nc.tensor.matmul(
                                psc[n][:], w[:, ksl, psl(n)],
                                a[:, ksl, bsl(b)],
                                start=(a is mhh and w is wch and k == 0),
                                stop=(a is mhl and k == MK // 2 - 1),
                                perf_mode=DR)
                for n in range(DK):
                    nc.scalar.activation(out=ctxz[:, n, bsl(b)],
                                         in_=psc[n][:],
                                         func=AF.Identity,
                                         bias=bcT[:, n:n + 1], scale=XS / WS)
            for b in range(NB):
                nc.vector.tensor_copy(out=ctxh[:, :, bsl(b)],
                                      in_=ctxz[:, :, bsl(b)])
                nc.gpsimd.tensor_sub(out=ctxl[:, :, bsl(b)],
                                     in0=ctxz[:, :, bsl(b)],
                                     in1=ctxh[:, :, bsl(b)])

            def rmsnorm_a(b):
                # phase A: sq = fp8(x*x) on ACT (no PE instruction, so it
                # can be emitted right after the stream's evacs).
                sq = sqp.tile([P, DK, BS], FP8, tag="sq")
                nc.scalar.activation(out=sq[:], in_=x[:, :, bsl(b)],
                                     func=AF.Square)
                return sq

            def sq_alloc():
                sq = sqp.tile([P, DK, BS], FP8, tag="sq")
                return sq

            def sq_half(b, n, sq):
                nc.scalar.activation(out=sq[:, n], in_=x[:, n, bsl(b)],
                                     func=AF.Square)

            def rmsnorm_b(b, sq, pool=False):
                # phase B: ss = DoubleRow ones-matmul (emit where sq is
                # already done, or PE head-of-line blocks); sqrt on ACT;
                # binv = XS*rsqrt via DVE recip -> bf16; z = x*binv bf16;
                # hi = fp8(z); lo = fp8(z - hi), per k-pair so the first
                # matmul terms unblock as early as possible.
                bs = bsl(b)
                z = norm.tile([P, DK, BS], BF16, tag="z")
                binv = norm.tile([P, BS], F32, tag="binv")
                binvb = norm.tile([P, BS], BF16, tag="binvb")
                srt = norm.tile([P, BS], F32, tag="srt")
                xh = xhh.tile([P, DK, BS], FP8, tag="xh")
                xl = xhl.tile([P, DK, BS], FP8, tag="xl")
                ss = ps_main.tile([P, BS], F32, tag="m")
                for k in range(DK // 2):
                    nc.tensor.matmul(ss[:], ones8[:],
                                     sq[:, 2 * k:2 * k + 2, :],
                                     start=(k == 0), stop=(k == DK // 2 - 1),
                                     perf_mode=DR)
                nc.scalar.activation(out=srt[:], in_=ss[:], func=AF.Sqrt,
                                     bias=eps_t8[:],
                                     scale=1.0 / (D * XS * XS))
                nc.vector.reciprocal_approx_fast(out=binv[:], in_=srt[:])
                eng = nc.gpsimd if pool else nc.vector
                eng.tensor_copy(out=binvb[:], in_=binv[:])
                for k in range(DK // 2):
                    ksl = slice(2 * k, 2 * k + 2)
                    for d in (2 * k, 2 * k + 1):
                        eng.tensor_mul(out=z[:, d], in0=x[:, d, bs],
                                       in1=binvb[:])
                    eng.tensor_copy(out=xh[:, ksl], in_=z[:, ksl])
                    eng.tensor_sub(out=xl[:, ksl], in0=z[:, ksl],
                                   in1=xh[:, ksl])
                return xh, xl

            for i in range(L):
                if i == L - 1:
                    nc.sync.dma_start(out=woh[:], in_=woh_d.ap())
                    nc.sync.dma_start(out=wol[:], in_=wol_d.ap())
                wsah = wpool.tile([P, DK, D], FP8, tag="wsah")
                nc.sync.dma_start(out=wsah[:], in_=wsah_d.ap()[i])
                wcah = wpool.tile([P, DK, D], FP8, tag="wcah")
                nc.sync.dma_start(out=wcah[:], in_=wcah_d.ap()[i])
                wsal = wpool.tile([P, DK, D], FP8, tag="wsal")
                nc.sync.dma_start(out=wsal[:], in_=wsal_d.ap()[i])
                wcal = wpool.tile([P, DK, D], FP8, tag="wcal")
                nc.sync.dma_start(out=wcal[:], in_=wcal_d.ap()[i])
                wgh = wpool.tile([P, DK, F], FP8, tag="wgh")
                nc.sync.dma_start(out=wgh[:], in_=wgh_d.ap()[i])
                wgl = wpool.tile([P, DK, F], FP8, tag="wgl")
                nc.sync.dma_start(out=wgl[:], in_=wgl_d.ap()[i])
                wuh = wpool.tile([P, DK, F], FP8, tag="wuh")
                nc.sync.dma_start(out=wuh[:], in_=wuh_d.ap()[i])
                wul = wpool.tile([P, DK, F], FP8, tag="wul")
                nc.sync.dma_start(out=wul[:], in_=wul_d.ap()[i])
                wdh = wpool.tile([P, FK, D], FP8, tag="wdh")
                nc.sync.dma_start(out=wdh[:], in_=wdh_d.ap()[i])
                wdl = wpool.tile([P, FK, D], FP8, tag="wdl")
                nc.sync.dma_start(out=wdl[:], in_=wdl_d.ap()[i])

                # ---- skewed two-stream layer body ----
                # Emission (= PE SEQ) order staggers the b0/b1 streams so
                # each serial norm / gu-split chain on ACT/DVE/Pool drains
                # under the OTHER stream's matmuls:
                #   sa/ca b0 | sa/ca b1      (ffn-norm b0 in its shadow)
                #   g/u  b0  | g/u  b1       (ffn-norm b1 / gu-gen b0)
                #   down b0  | down b1       (gu-gen b1 / next sa-norm b0)
                # Next-layer sa-norms are emitted right after each stream's
                # down evacs, so they drain during the remaining stream work.
                def ca_pass(b):
                    pss = []
                    for n in range(DK):
                        ps = ps_main.tile([P, BS], F32, tag="m")
                        pss.append(ps)
                        mm3(ps, ctxh[:, :, bsl(b)], ctxl[:, :, bsl(b)],
                            wcah, wcal, DK, n, stop=False)
                    return pss

                def sa_pass(b, pss, sq=None):
                    xh = xh_sa[b]
                    for n in range(DK):
                        mm3(pss[n], xh[0], xh[1], wsah, wsal, DK, n,
                            start=False)
                        xsl = x[:, n, bsl(b)]
                        nc.vector.scalar_tensor_tensor(
                            out=xsl, in0=pss[n][:], scalar=1.0 / (WS * XS),
                            in1=xsl, op0=ALU.mult, op1=ALU.add)
                        if sq is not None:
                            sq_half(b, n, sq)

                def gu_stage(b, mid=None):
                    guh = gup.tile([P, FK, BS], FP8, tag="guh")
                    for f in range(FK):
                        if f == FK // 2 and mid is not None:
                            mid()
                        psg = ps_main.tile([P, BS], F32, tag="m")
                        psu = ps_main.tile([P, BS], F32, tag="m")
                        mm3(psg, xh_ffn[b][0], xh_ffn[b][1], wgh, wgl, DK, f)
                        mm3(psu, xh_ffn[b][0], xh_ffn[b][1], wuh, wul, DK, f)
                        sg = sgp.tile([P, BS], BF16, tag="sg")
                        nc.scalar.activation(out=sg[:], in_=psg[:],
                                             func=AF.Silu,
                                             scale=1.0 / (WS * XS))
                        # u evacs on ACT so the gub product is an all-bf16
                        # SBUF DVE mul (2x fast mode), then split to fp8
                        ub = sgp.tile([P, BS], BF16, tag="ub")
                        nc.scalar.activation(out=ub[:], in_=psu[:],
                                             func=AF.Identity,
                                             scale=GS / (WS * XS))
                        gub = sgp.tile([P, BS], BF16, tag="gub")
                        nc.vector.tensor_mul(out=gub[:], in0=sg[:],
                                             in1=ub[:])
                        heng = nc.gpsimd if b == 0 else nc.vector
                        heng.tensor_copy(out=guh[:, f], in_=gub[:])
                    return guh, None, sg

                def down(b, guh, gul, sq=None):
                    # 2-term: gu_lo dropped (see gu_stage)
                    for n in range(DK):
                        ps = ps_main.tile([P, BS], F32, tag="m")
                        cs = psl(n)
                        first = True
                        for w in (wdh, wdl):
                            for k in range(FK // 2):
                                ksl = slice(2 * k, 2 * k + 2)
                                last = (w is wdl) and (k == FK // 2 - 1)
                                nc.tensor.matmul(
                                    ps[:], w[:, ksl, cs], guh[:, ksl, :],
                                    start=first, stop=last, perf_mode=DR)
                                first = False
                        xsl = x[:, n, bsl(b)]
                        nc.vector.scalar_tensor_tensor(
                            out=xsl, in0=ps[:], scalar=1.0 / (WS * GS),
                            in1=xsl, op0=ALU.mult, op1=ALU.add)
                        if sq is not None:
                            sq_half(b, n, sq)

                sq0 = sq_alloc()
                sq1 = sq_alloc()
                if i == 0:
                    xh_sa = xh0t
                    pss0 = ca_pass(0)
                    sa_pass(0, pss0, sq=sq0)
                    pss1 = ca_pass(1)
                    sa_pass(1, pss1, sq=sq1)
                    xh_ffn = [rmsnorm_b(0, sq0), None]
                else:
                    # xh_sa[0] norm chain (emitted in the prev layer tail)
                    # drains under ca_pass(0); xh_sa[1] under sa_pass(0)
                    # + ca_pass(1).
                    pss0 = ca_pass(0)
                    xh_sa[1] = rmsnorm_b(1, sqn1)
                    sa_pass(0, pss0, sq=sq0)
                    pss1 = ca_pass(1)
                    sa_pass(1, pss1, sq=sq1)
                    xh_ffn = [rmsnorm_b(0, sq0), None]
                xh_ffn[1] = rmsnorm_b(1, sq1)
                if i == 0:
                    preload_act(AF.Silu, xh_ffn[0][0][:, 0, 0:1])
                gu0 = gu_stage(0)
                gu1 = gu_stage(1)
                if i < L - 1:
                    sqn0 = sq_alloc()
                    down(0, gu0[0], gu0[1], sq=sqn0)
                    # b0's next sa-norm: sq halves landed inline with the
                    # down(0) evacs, so its ss barely stalls PE here and the
                    # DVE chain drains under down(1).
                    xh_sa = [rmsnorm_b(0, sqn0), None]
                else:
                    down(0, gu0[0], gu0[1])
                    # Final-norm sqrt table, pinned while PE runs down b1.
                    preload_act(AF.Sqrt, gu1[2][:, 0:1])
                down(1, gu1[0], gu1[1])
                if i < L - 1:
                    sqn1 = rmsnorm_a(1)

            # ---- final rmsnorm folded into the unembed output side ----
            # logits[s, v] = (x8[:, s] . wout[:, v]) * bcol[s] with
            # x8 = split-fp8(XS * x) and bcol = rsqrt(mean x^2 + eps)/(WS*XS)
            # (batch-major [128,1] via tiny sq-matmuls, no transpose).
            bcols = []
            for b in range(NB):
                bs = bsl(b)
                z8 = norm.tile([P, DK, BS], BF16, tag="z8")
                nc.scalar.activation(out=z8[:], in_=x[:, :, bs],
                                     func=AF.Identity, scale=XS)
                nc.vector.tensor_copy(out=x8h[:, :, bs], in_=z8[:])
                nc.gpsimd.tensor_sub(out=x8l[:, :, bs], in0=z8[:],
                                     in1=x8h[:, :, bs])
                sq = sqp.tile([P, DK, BS], FP8, tag="sq")
                nc.scalar.activation(out=sq[:], in_=x[:, :, bs],
                                     func=AF.Square)
                for t in range(BS // P):
                    bt = b * (BS // P) + t
                    ssb = ps_ty.tile([P, 1], F32, tag="tiny")
                    for k in range(DK // 2):
                        nc.tensor.matmul(
                            ssb[:], sq[:, 2 * k:2 * k + 2, t * P:(t + 1) * P],
                            onecol8[:], start=(k == 0),
                            stop=(k == DK // 2 - 1), perf_mode=DR)
                    srtc = tinyp.tile([P, 1], F32, tag="srtc")
                    # srt = WS*XS*sqrt(ms+eps); bcol = rsqrt/(WS*XS)
                    nc.scalar.activation(out=srtc[:], in_=ssb[:],
                                         func=AF.Sqrt, bias=eps_tw[:],
                                         scale=WS * XS * WS * XS / D)
                    bcol = tinyp.tile([P, 1], F32, tag="bcol")
                    nc.vector.reciprocal_approx_fast(out=bcol[:],
                                                     in_=srtc[:])
                    bcols.append(bcol)

            for bt in range(BT):
                for v in range(VK):
                    ps = ps_main.tile([P, BS], F32, tag="m")
                    first = True
                    for (a, w) in ((x8h, woh), (x8h, wol), (x8l, woh)):
                        for k in range(DK // 2):
                            ksl = slice(2 * k, 2 * k + 2)
                            last = (a is x8l) and (k == DK // 2 - 1)
                            nc.tensor.matmul(
                                ps[:], a[:, ksl, psl(bt)],
                                w[:, ksl, bsl(v)],
                                start=first, stop=last, perf_mode=DR)
                            first = False
                    if v % 2 == 0:
                        st = stage.tile([P, 2, BS], BF16, tag="st")
                        nc.scalar.activation(out=st[:, 0], in_=ps[:],
                                             func=AF.Identity,
                                             scale=bcols[bt][:])
                    else:
                        # in1 ignored by bypass; must be SBUF, not PSUM
                        nc.vector.scalar_tensor_tensor(
                            out=st[:, 1], in0=ps[:], scalar=bcols[bt][:],
                            in1=x[:, 0, 0:BS], op0=ALU.mult, op1=ALU.bypass)
                        # one wide DMA per v-pair halves HWDGE setups
                        nc.sync.dma_start(
                            out=out_d.ap()[psl(bt),
                                           (v - 1) * BS:(v + 1) * BS],
                            in_=st[:])

    nc.finalize()
    return nc


def _get_nc():
    if "nc" not in _CACHE:
        _CACHE["nc"] = _build_nc()
    return _CACHE["nc"]


def _bf16(a):
    return np.ascontiguousarray(a).astype(ml_dtypes.bfloat16)


_F8NP = mybir.dt.np(FP8)


def _split8(a):
    hi = np.ascontiguousarray(a).astype(_F8NP)
    lo = np.ascontiguousarray(a - hi.astype(np.float64)).astype(_F8NP)
    return hi, lo


def _kxn(w):
    # [K, N] f64 -> [P, K//P, N]: [:, k, n0:n0+128] is a natural lhsT chunk.
    K, N = w.shape
    return np.ascontiguousarray(w.reshape(K // P, P, N).transpose(1, 0, 2))


def _dim_major(a):
    # [B, K] -> per-core [core, P, K//P, BC]
    K = a.shape[1]
    return a.reshape(N_CORES, BC, K // P, P).transpose(0, 3, 2, 1)


def _prep_inputs(inputs):
    f = {k: np.asarray(v) for k, v in inputs.items()}
    prev = f["prev_tokens"].reshape(-1).astype(np.int64)
    emb = f["emb"].astype(np.float64)
    mhf = f["main_hidden"].reshape(B, MD).astype(np.float64)

    # Layer-0 sa xhat: exact f64 normalization of the embedding TABLE,
    # split to fp8 per-row, then gathered (split before gather == after).
    nemb = emb * (XS / np.sqrt((emb * emb).mean(axis=1, keepdims=True) + EPS))
    nembh, nembl = _split8(nemb)
    x0c = _dim_major(emb[prev])
    xh0hc = _dim_major(nembh.astype(np.float64)[prev])
    xh0lc = _dim_major(nembl.astype(np.float64)[prev])
    mhc = _dim_major(mhf)

    f64 = lambda k: f[k].astype(np.float64)
    sa_n, sa_v, sa_o = f64("sa_n"), f64("sa_v"), f64("sa_o")
    ca_v, ca_o = f64("ca_v"), f64("ca_o")
    ffn_n, w_g, w_u, w_d = f64("ffn_n"), f64("w_g"), f64("w_u"), f64("w_d")

    def wsplit(ws):
        hs, ls = zip(*(_split8(w) for w in ws))
        return np.stack(hs), np.stack(ls)

    wsah, wsal = wsplit([_kxn(WS * (sa_n[i][:, None] * sa_v[i]) @ sa_o[i])
                         for i in range(L)])
    wcah, wcal = wsplit([_kxn(WS * ca_v[i] @ ca_o[i]) for i in range(L)])
    wgh, wgl = wsplit([_kxn(WS * ffn_n[i][:, None] * w_g[i])
                       for i in range(L)])
    wuh, wul = wsplit([_kxn(WS * ffn_n[i][:, None] * w_u[i])
                       for i in range(L)])
    wdh, wdl = wsplit([_kxn(WS * w_d[i]) for i in range(L)])
    woh, wol = _split8(_kxn(WS * f64("final_n")[:, None] * f64("Wout")))
    wch, wcl = _split8(_kxn(WS * f64("Wc")))
    bcv = np.ascontiguousarray(
        XS * f["bc"].astype(np.float64).reshape(DK, P).T)

    shared = {
        "wch": wch, "wcl": wcl, "bcv": bcv.astype(np.float32),
        "wsah": wsah, "wsal": wsal, "wcah": wcah, "wcal": wcal,
        "wgh": wgh, "wgl": wgl, "wuh": wuh, "wul": wul,
        "wdh": wdh, "wdl": wdl, "woh": woh, "wol": wol,
    }
    in_maps = []
    for c in range(N_CORES):
        m = dict(shared)
        m["x0"] = _bf16(x0c[c])
        m["mhh"], m["mhl"] = _split8(mhc[c])
        m["xh0h"] = xh0hc[c].astype(_F8NP)
        m["xh0l"] = xh0lc[c].astype(_F8NP)
        in_maps.append(m)
    return in_maps


def _run(inputs, trace=False, **kw):
    nc = _get_nc()
    in_maps = _prep_inputs(inputs)
    res = bass_utils.run_bass_kernel_spmd(
        nc, in_maps, core_ids=list(range(N_CORES)), trace=trace, **kw)
    out = np.concatenate([res.results[c]["out"] for c in range(N_CORES)],
                         axis=0)
    return out.astype(np.float32), res


def kernel(**inputs) -> np.ndarray:
    out, _ = _run(inputs, trace=False)
    return out
